# revision 9
# baseline (speedup 1.0000x reference)
"""Local causal (sliding-window) attention kernel for Trainium2, SPMD over 8 cores.

Problem: states [4, 4096, 1024] f32; q/k/v = states @ W*.T + b*; each query t
attends keys t-8..t (window=8), softmax over valid positions, out = attn @ v.

Sharding: data-parallel, 8 shards = 4 batches x 2 sequence halves (2048 queries
each). The host supplies each shard's states pre-transposed to [H, 2056] with an
8-row halo (zero-padded at sequence start; masked out via the additive mask).

Score reformulation (saves one full GEMM): q.k = x_t^T A x_k + g.x_t + w.x_k
+ c0 with A = (Wq/sqrt(H))^T Wk precomputed on host. The device computes
Y = A @ X (one GEMM) instead of both Q and K projections; X itself serves as
the score lhsT. The rank-1 terms u[k] = w.x_k and gx[t] = g.x_t are host
GEMVs accumulated into the score PSUM via two tiny K=1 matmuls; c0 is folded
into the additive masks.

Per-core device plan (bf16 matmuls, f32 PSUM):
  - Y[H,2056] (halo cols from host), v[2048+tail,H] row-major (tail rows from
    host). All ragged matmul chunks eliminated.
  - Attention per 128-query tile j: S[128,136] = X_q^T Y_span (8 accum matmuls
    + 2 rank-1); masked softmax along free dim with fused exp+rowsum; P
    transposed via PE identity matmul; out = P @ V_span (136-contraction split
    128+8); 1/rowsum applied on the PSUM->SBUF copy.
"""

import numpy as np
import ml_dtypes

import concourse.bacc as bacc
import concourse.mybir as mybir
import concourse.tile as tile
from concourse.bass_utils import run_bass_kernel_spmd

B, T, H = 4, 4096, 1024
NCORES = 8
TC = T // 2            # queries per core
HALO = 8               # window size
TH = TC + HALO         # shard cols incl. halo
SPAN = 128 + HALO      # key span per 128-query tile
NT = TC // 128         # query tiles per core
HC = H // 128          # 128-row chunks of H
F32 = mybir.dt.float32
BF16 = mybir.dt.bfloat16
BF = ml_dtypes.bfloat16
AF = mybir.ActivationFunctionType

_cache = {}


def _emit(nc, tc, aps, pools):
    (x_d, a_d, wv_d, bv_d, m0_d, mr_d, id_d, u_d, on_d, yh_d,
     vt_d, out_d) = aps
    consts, xw, acts, psP, psS, psT, psO, attn = pools

    bv_t = consts.tile([128, H], F32, tag="bv", name="bv_t")
    m0_t = consts.tile([128, SPAN], F32, tag="m0", name="m0_t")
    mr_t = consts.tile([128, SPAN], F32, tag="mr", name="mr_t")
    id_t = consts.tile([128, 128], BF16, tag="id", name="id_t")
    u_t = consts.tile([1, TH], BF16, tag="u", name="u_t")
    on_t = consts.tile([1, 128], BF16, tag="on", name="on_t")

    xt = [xw.tile([128, TH], BF16, tag=f"x{c}", name=f"x{c}") for c in range(HC)]
    at = [xw.tile([128, H], BF16, tag=f"a{c}", name=f"a{c}") for c in range(HC)]
    wvt = [xw.tile([128, H], BF16, tag=f"wv{c}", name=f"wv{c}") for c in range(HC)]
    yt = [acts.tile([128, TH], BF16, tag=f"y{c}", name=f"y{c}") for c in range(HC)]
    vt = [acts.tile([128, H], BF16, tag=f"v{j}", name=f"v{j}")
          for j in range(NT)]
    vtail = acts.tile([HALO, H], BF16, tag="vtail", name="vtail")

    # DMA issue order = data-need order. First Y group needs x seg0 + a;
    # first V group additionally wv; attention group 0 needs consts.
    for c in range(HC):   # x cols for the first two v tiles
        nc.gpsimd.dma_start(xt[c][:, 0:264], x_d[c * 128:(c + 1) * 128, 0:264])
    for c in range(HC):
        nc.gpsimd.dma_start(wvt[c][:, 0:512],
                            wv_d[c * 128:(c + 1) * 128, 0:512])
    for c in range(HC):
        nc.gpsimd.dma_start(wvt[c][:, 512:H],
                            wv_d[c * 128:(c + 1) * 128, 512:H])
    for c in range(HC):   # rest of the first x segment
        nc.gpsimd.dma_start(xt[c][:, 264:HALO + 512],
                            x_d[c * 128:(c + 1) * 128, 264:HALO + 512])
    for c in range(HC):
        nc.gpsimd.dma_start(at[c][:], a_d[c * 128:(c + 1) * 128, :])
    nc.gpsimd.dma_start(bv_t[:], bv_d[:])
    nc.gpsimd.dma_start(m0_t[:], m0_d[:])
    nc.gpsimd.dma_start(mr_t[:], mr_d[:])
    nc.gpsimd.dma_start(id_t[:], id_d[:])
    nc.gpsimd.dma_start(u_t[:], u_d[:])
    nc.gpsimd.dma_start(on_t[:], on_d[:])
    for c in range(HC):   # Y halo cols from host
        nc.gpsimd.dma_start(yt[c][:, 0:HALO], yh_d[c * 128:(c + 1) * 128, :])
    for seg in range(1, TC // 512):   # remaining x column segments
        lo = HALO + seg * 512
        for c in range(HC):
            nc.gpsimd.dma_start(xt[c][:, lo:lo + 512],
                                x_d[c * 128:(c + 1) * 128, lo:lo + 512])
    nc.gpsimd.dma_start(vtail[:], vt_d[:])

    def emit_y(t4):
        off = HALO + t4 * 512
        for hc in range(HC):
            ps = psP.tile([128, 512], F32, tag="ps", name="psy")
            for c in range(HC):
                nc.tensor.matmul(
                    ps[:], at[c][:, hc * 128:(hc + 1) * 128],
                    xt[c][:, off: off + 512],
                    start=(c == 0), stop=(c == HC - 1))
            nc.scalar.copy(yt[hc][:, off: off + 512], ps[:])

    def emit_v(j):
        for hh in range(2):
            ps = psP.tile([128, 512], F32, tag="ps", name="psv")
            for c in range(HC):
                nc.tensor.matmul(
                    ps[:], xt[c][:, j * 128: (j + 1) * 128],
                    wvt[c][:, hh * 512:(hh + 1) * 512],
                    start=(c == 0), stop=(c == HC - 1))
            nc.vector.tensor_add(
                vt[j][:, hh * 512:(hh + 1) * 512], ps[:],
                bv_t[:, hh * 512:(hh + 1) * 512])

    def emit_attn(j):
        s_ps = psS.tile([128, SPAN], F32, tag="s", name="s_ps")
        for c in range(HC):
            nc.tensor.matmul(
                s_ps[:], xt[c][:, HALO + j * 128: HALO + (j + 1) * 128],
                yt[c][:, j * 128: j * 128 + SPAN],
                start=(c == 0), stop=False)
        nc.tensor.matmul(s_ps[:], on_t[:, 0:128],
                         u_t[:, j * 128: j * 128 + SPAN],
                         start=False, stop=True)
        s_sb = attn.tile([128, SPAN], F32, tag="ssb", name="s_sb")
        nc.vector.tensor_add(s_sb[:], s_ps[:],
                             (m0_t if j == 0 else mr_t)[:])
        negmax = attn.tile([128, 1], F32, tag="nm", name="negmax")
        nc.vector.reduce_max(negmax[:], s_sb[:],
                             axis=mybir.AxisListType.X, negate=True)
        p_bf = attn.tile([128, SPAN], BF16, tag="p", name="p_bf")
        rowsum = attn.tile([128, 1], F32, tag="rs", name="rowsum")
        nc.scalar.activation(p_bf[:], s_sb[:], AF.Exp,
                             bias=negmax[:], scale=1.0,
                             accum_out=rowsum[:])
        rinv = attn.tile([128, 1], F32, tag="ri", name="rinv")
        nc.vector.reciprocal(rinv[:], rowsum[:])

        pt_ps = psT.tile([128, 256], BF16, tag="pt", name="pt_ps")
        nc.tensor.transpose(pt_ps[:, 0:128], p_bf[:, 0:128], id_t[:])
        nc.tensor.transpose(pt_ps[:HALO, 128:256], p_bf[:, 128:SPAN], id_t[:])
        pta_sb = attn.tile([128, 128], BF16, tag="ptas", name="pta_sb")
        ptb_sb = attn.tile([HALO, 128], BF16, tag="ptbs", name="ptb_sb")
        nc.scalar.copy(pta_sb[:], pt_ps[:, 0:128])
        nc.vector.tensor_copy(ptb_sb[:], pt_ps[:HALO, 128:256])

        vnext = vtail if j == NT - 1 else vt[j + 1]
        out_sb = attn.tile([128, H], F32, tag="osb", name="out_sb")
        for hh in range(2):
            o_ps = psO.tile([128, 512], F32, tag="o", name="o_ps")
            nc.tensor.matmul(o_ps[:], pta_sb[:],
                             vt[j][:, hh * 512:(hh + 1) * 512],
                             start=True, stop=False)
            nc.tensor.matmul(o_ps[:], ptb_sb[:],
                             vnext[:HALO, hh * 512:(hh + 1) * 512],
                             start=False, stop=True)
            nc.scalar.activation(
                out_sb[:, hh * 512:(hh + 1) * 512], o_ps[:],
                AF.Copy, bias=0.0, scale=rinv[:])
        nc.sync.dma_start(out_d[j * 128:(j + 1) * 128, :], out_sb[:])

    # Interleave: after Y t-chunk t4 and v tiles 4*t4..4*t4+3, attention
    # tiles j <= 4*t4+2 have everything they need (span fits in Y cols
    # < 8+512*(t4+1); PV needs v[j+1]).
    emit_v(0)
    emit_v(1)
    emit_y(0)
    emit_v(2)
    emit_v(3)
    for j in range(0, 3):
        emit_attn(j)
    for t4 in range(1, TC // 512):
        emit_y(t4)
        for j in range(4 * t4, 4 * t4 + 4):
            emit_v(j)
        for j in range(4 * t4 - 1, 4 * t4 + 3):
            emit_attn(j)
    emit_attn(NT - 1)


def _build(loop_reps=None, trace_sim=False):
    key = ("nc", loop_reps, trace_sim)
    if key in _cache:
        return _cache[key]
    nc = bacc.Bacc("TRN2", target_bir_lowering=False, debug=False,
                   num_devices=NCORES)

    aps = (
        nc.dram_tensor("x", [H, TH], BF16, kind="ExternalInput").ap(),
        nc.dram_tensor("a", [H, H], BF16, kind="ExternalInput").ap(),
        nc.dram_tensor("wv", [H, H], BF16, kind="ExternalInput").ap(),
        nc.dram_tensor("bv", [128, H], F32, kind="ExternalInput").ap(),
        nc.dram_tensor("m0", [128, SPAN], F32, kind="ExternalInput").ap(),
        nc.dram_tensor("mr", [128, SPAN], F32, kind="ExternalInput").ap(),
        nc.dram_tensor("ident", [128, 128], BF16, kind="ExternalInput").ap(),
        nc.dram_tensor("u", [1, TH], BF16, kind="ExternalInput").ap(),
        nc.dram_tensor("ones", [1, 128], BF16, kind="ExternalInput").ap(),
        nc.dram_tensor("yhalo", [H, HALO], BF16, kind="ExternalInput").ap(),
        nc.dram_tensor("vtail", [HALO, H], BF16, kind="ExternalInput").ap(),
        nc.dram_tensor("out", [TC, H], F32, kind="ExternalOutput").ap(),
    )

    with tile.TileContext(nc, trace_sim=trace_sim) as tc:
        with (
            tc.tile_pool(name="consts", bufs=1) as consts,
            tc.tile_pool(name="xw", bufs=1) as xw,
            tc.tile_pool(name="acts", bufs=1) as acts,
            tc.tile_pool(name="psP", bufs=2, space="PSUM") as psP,
            tc.tile_pool(name="psS", bufs=2, space="PSUM") as psS,
            tc.tile_pool(name="psT", bufs=1, space="PSUM") as psT,
            tc.tile_pool(name="psO", bufs=3, space="PSUM") as psO,
            tc.tile_pool(name="attn", bufs=3) as attn,
        ):
            pools = (consts, xw, acts, psP, psS, psT, psO, attn)
            if loop_reps:
                with tc.For_i(0, loop_reps, 1):
                    _emit(nc, tc, aps, pools)
            else:
                _emit(nc, tc, aps, pools)

    nc.compile()
    _cache[key] = nc
    return nc


def _host_inputs(states, Wq, bq, Wk, bk, Wv, bv):
    """Shared (per-run) host-side tensor prep."""
    scale = 1.0 / np.sqrt(H)
    Wq = np.asarray(Wq, np.float32)
    Wk = np.asarray(Wk, np.float32)
    Wv = np.asarray(Wv, np.float32)
    bq = np.asarray(bq, np.float32)
    bk = np.asarray(bk, np.float32)
    bv = np.asarray(bv, np.float32)
    Wqs = Wq * scale
    # A = Wqs.T @ Wk ; device lhsT layout needs A.T = Wk.T @ Wqs
    at_h = np.ascontiguousarray(Wk.T @ Wqs).astype(BF)
    # per-key rank-1 vector; per-query term and constants cancel in softmax
    wt_h = Wk.T @ (bq * scale)
    wv_h = np.ascontiguousarray(Wv.T).astype(BF)
    bv_h = np.ascontiguousarray(np.broadcast_to(bv, (128, H)))
    m = np.arange(128)[:, None]
    n = np.arange(SPAN)[None, :]
    band = (n >= m) & (n <= m + HALO)
    mr_h = np.where(band, 0.0, -30000.0).astype(np.float32)
    m0_h = np.where(band & (n >= HALO), 0.0, -30000.0).astype(np.float32)
    id_h = np.eye(128).astype(BF)
    on_h = np.ones((1, 128), dtype=BF)
    return at_h, wt_h, wv_h, bv_h, m0_h, mr_h, id_h, on_h, bv


def _shard_maps(states, hosts):
    at_h, wt_h, wv_h, bv_h, m0_h, mr_h, id_h, on_h, bv = hosts
    a_f = at_h.astype(np.float32)      # [hin, hout] = A.T in bf16 precision
    wv_f = wv_h.astype(np.float32)
    in_maps = []
    for i in range(NCORES):
        b, hf = i // 2, i % 2
        xs = np.zeros((TH, H), np.float32)
        if hf == 0:
            xs[HALO:] = states[b, 0:TC]
        else:
            xs[:] = states[b, TC - HALO: 2 * TC]
        x_h = np.ascontiguousarray(xs.T).astype(BF)   # [H, TH]
        x_f = x_h.astype(np.float32)
        u_h = (wt_h @ x_f).astype(BF).reshape(1, TH)
        yh_h = (a_f.T @ x_f[:, :HALO]).astype(BF)      # [H, 8]
        vtail_h = (x_f[:, TC:].T @ wv_f + bv).astype(BF)  # [8, H]
        in_maps.append({
            "x": x_h, "a": at_h, "wv": wv_h, "bv": bv_h,
            "m0": (m0_h if hf == 0 else mr_h), "mr": mr_h, "ident": id_h,
            "u": u_h, "ones": on_h, "yhalo": yh_h,
            "vtail": vtail_h,
        })
    return in_maps


def kernel(states, Wq, bq, Wk, bk, Wv, bv, window):
    assert int(window) == HALO
    states = np.asarray(states, np.float32)
    nc = _build()
    hosts = _host_inputs(states, Wq, bq, Wk, bk, Wv, bv)
    in_maps = _shard_maps(states, hosts)
    res = run_bass_kernel_spmd(nc, in_maps, list(range(NCORES)))
    out = np.empty((B, T, H), np.float32)
    for i in range(NCORES):
        b, hf = i // 2, i % 2
        out[b, hf * TC:(hf + 1) * TC] = res.results[i]["out"]
    return out
